# revision 33
# baseline (speedup 1.0000x reference)
"""Causal single-head attention (B=4, S=4096, D_MODEL=1024, D_K=D_V=128)
distributed over 8 TRN2 NeuronCores.

Sharding: batch (4) x interleaved query-tile parity (2) = 8 cores.
Core c handles batch b=c//2, parity p=c%2; its local q-tile i (16 tiles of
128 queries) is global q-tile T = 2*i + p.  The causal workload per-core is
identical (sum over i of (2i+2) key-tiles), so one SPMD program serves all
cores; the parity difference lives in two host-supplied [128,128] masks.

v2: each core projects only its parity's half of the keys (even core: keys
[0:2048), odd: [2048:4096)) and the projected K^T / V tiles are exchanged
within each batch pair by two AllGather collectives (0.5 MB each), halving
both the k/v HBM traffic and the K/V projection FLOPs.

Per-core compute layout ("all transposed", no PE transposes):
  - projections produce QT [dk=128, q], KT [dk=128, keys] (N=512 matmuls)
    and V [keys, dv] (N=128 matmuls)
  - scores ST = [keys=128, q<=512] tiles, two key-tiles per PSUM tile so
    one ACT exp covers a pair (ACT ops pay (N+352)/1.2 ns)
  - softmax: exp on ScalarE (scores bounded ~|z|<3: no max subtraction),
    causal via binary mask multiplies on VectorE
  - PV: matmul(lhsT=P_tile [keys,128q], rhs=V_aug [keys,129]) -> output in
    natural [q, dv] layout with the softmax denominator in column 128;
    normalization = DVE reciprocal + tensor_scalar_mul.
"""

import math
import numpy as np
import ml_dtypes

import concourse.bass as bass
import concourse.mybir as mybir
from concourse import bacc, tile
from concourse.bass_utils import run_bass_kernel_spmd

BF16NP = ml_dtypes.bfloat16
F32 = mybir.dt.float32
BF16 = mybir.dt.bfloat16

B = 4
S = 4096
DM = 1024
DK = 128
DV = 128
SQ = 2048          # queries per core
NQT = 16           # local q-tiles of 128
NMC = DM // 128    # 8 contraction chunks for projections
MAXKT = S // 128   # 32 key tiles
NCH = SQ // 512    # 4 q-chunks of 512
SK = S // 2        # keys projected locally per core (v2)

MODE = "v6"        # v15: full local K/V | v2/v3/v4: collective variants | v6: local K/V, interleaved attention

LAST_RESULTS = None
_NC_CACHE = {}


def build_nc(mode="v2", vt=False):
    collective = mode == "v2"
    nkeys = SK if collective else S

    nc = bacc.Bacc(None, target_bir_lowering=False, num_devices=8)

    qT = nc.declare_dram_parameter("qT", [DM, SQ], BF16, isOutput=False)
    kT = nc.declare_dram_parameter("kT", [DM, nkeys], BF16, isOutput=False)
    vT = nc.declare_dram_parameter("vT", [DM, nkeys], BF16, isOutput=False)
    wq = nc.declare_dram_parameter("wq", [128, NMC * DK], BF16, isOutput=False)
    wk = nc.declare_dram_parameter("wk", [128, NMC * DK], BF16, isOutput=False)
    wv = nc.declare_dram_parameter("wv", [128, NMC * DV], BF16, isOutput=False)
    mska = nc.declare_dram_parameter("mska", [128, 128], BF16, isOutput=False)
    mskb = nc.declare_dram_parameter("mskb", [128, 128], BF16, isOutput=False)
    eye = nc.declare_dram_parameter("eye", [128, 128], BF16, isOutput=False)
    out = nc.declare_dram_parameter("out", [SQ, DV], F32, isOutput=True)

    Exp = mybir.ActivationFunctionType.Exp

    with tile.TileContext(nc) as tc:
        with (
            tc.tile_pool(name="const", bufs=1) as constp,
            tc.tile_pool(name="stream", bufs=3) as streamp,
            tc.tile_pool(name="big", bufs=1) as bigp,
            tc.tile_pool(name="ptp", bufs=2) as ptp,
            tc.tile_pool(name="outp", bufs=4) as outp,
            tc.tile_pool(name="dram", bufs=1, space="DRAM") as dramp,
            tc.tile_pool(name="ps", bufs=2, space="PSUM") as psp,
            tc.tile_pool(name="pst", bufs=2, space="PSUM") as pstp,
            tc.tile_pool(name="pso", bufs=2, space="PSUM") as psop,
        ):
            # ---- weights (host pre-tiled to [128, mc*128] contiguous) ----
            wk_sb = constp.tile([128, NMC, DK], BF16)
            nc.sync.dma_start(wk_sb[:], wk.rearrange("p (mc d) -> p mc d", d=DK))
            wv_sb = constp.tile([128, NMC, DV], BF16)
            nc.sync.dma_start(wv_sb[:], wv.rearrange("p (mc d) -> p mc d", d=DV))
            wq_sb = constp.tile([128, NMC, DK], BF16)
            nc.sync.dma_start(wq_sb[:], wq.rearrange("p (mc d) -> p mc d", d=DK))

            # ---- PE warm-up: keep TensorE busy during the input-DMA lead-in
            # so HAM is at 8/8 when the first real matmul's data lands ----
            warm = constp.tile([128, 256], BF16)
            nc.vector.memset(warm[:], 0.0)
            wps = psp.tile([128, 256], F32, tag="projps")
            for _ in range(50):
                nc.tensor.matmul(wps[:], warm[:, 0:128], warm[:], start=True, stop=True)

            # ---- persistent activations ----
            QT = bigp.tile([128, SQ], BF16)
            KT = bigp.tile([128, S], BF16)
            VA = bigp.tile([128, MAXKT, DV + 2], BF16)

            # ---- projection helper: streams src in quarters of 1024 cols ----
            def proj_quarters(src_dram, n_cols):
                for qtr in range(n_cols // 1024):
                    tin = streamp.tile([128, NMC, 1024], BF16, tag="instream")
                    nc.sync.dma_start(
                        tin[:],
                        src_dram[:, qtr * 1024:(qtr + 1) * 1024].rearrange(
                            "(mc p) c -> p mc c", p=128
                        ),
                    )
                    yield qtr, tin

            def proj512(w_sb, tin, half, dst_sb_slice):
                ps = psp.tile([128, 512], F32, tag="projps")
                for m in range(NMC):
                    nc.tensor.matmul(
                        ps[:], w_sb[:, m, :], tin[:, m, half * 512:(half + 1) * 512],
                        start=(m == 0), stop=(m == NMC - 1),
                    )
                nc.vector.tensor_copy(dst_sb_slice, ps[:])

            # ---- Q projection first (non-collective): measured best PE order.
            # First two chunks are 512 cols so the very first matmul's data
            # arrives sooner. ----
            if not collective:
                cb = 0
                for w in (512, 512, 1024):
                    tin = streamp.tile([128, NMC, w], BF16, tag="instream",
                                       name=f"qin{cb}")
                    nc.sync.dma_start(
                        tin[:],
                        qT[:, cb * 512:cb * 512 + w].rearrange(
                            "(mc p) c -> p mc c", p=128
                        ),
                    )
                    for half in range(w // 512):
                        proj512(wq_sb, tin, half, QT[:, (cb + half) * 512:(cb + half + 1) * 512])
                    cb += w // 512

            # ---- K projection (local keys) ----
            if collective:
                KT_loc = bigp.tile([128, SK], BF16)
                k_dst = KT_loc
            else:
                k_dst = KT
            for qtr, tin in proj_quarters(kT, nkeys):
                for half in range(2):
                    cb = qtr * 2 + half
                    proj512(wk_sb, tin, half, k_dst[:, cb * 512:(cb + 1) * 512])

            if collective:
                cc_in_k = dramp.tile([128, SK], BF16)
                cc_out_k = dramp.tile([2, 128, SK], BF16)
                nc.sync.dma_start(cc_in_k[:], KT_loc[:])
                nc.gpsimd.collective_compute(
                    "AllGather",
                    mybir.AluOpType.bypass,
                    replica_groups=[[0, 1], [2, 3], [4, 5], [6, 7]],
                    ins=[cc_in_k[:]],
                    outs=[cc_out_k[:]],
                )

            # ---- V projection (local keys, natural [keys, dv] layout) ----
            nc.vector.memset(VA[:], 1.0)  # ones column at [:, :, DV]
            nloc_kt = nkeys // 128
            if collective:
                V_loc = bigp.tile([128, nloc_kt, DV], BF16)
            if vt and not collective:
                # N=512 VT projection + PE-mode transpose into VA tiles,
                # interleaved per 512-key chunk so HAM stays warm
                eye_sb = constp.tile([128, 128], BF16)
                nc.sync.dma_start(eye_sb[:], eye[:])
                VTS = bigp.tile([128, S], BF16)
                for qtr, tin in proj_quarters(vT, nkeys):
                    for half in range(2):
                        cb = qtr * 2 + half
                        proj512(wv_sb, tin, half, VTS[:, cb * 512:(cb + 1) * 512])
                        for sl in range(4):
                            kt_idx = cb * 4 + sl
                            tp = psop.tile([128, 128], BF16, tag="ops")
                            nc.tensor.transpose(
                                tp[:], VTS[:, kt_idx * 128:(kt_idx + 1) * 128], eye_sb[:]
                            )
                            nc.vector.tensor_copy(VA[:, kt_idx, 0:DV], tp[:])
            else:
                for qtr, tin in proj_quarters(vT, nkeys):
                    for sl in range(8):
                        kt_idx = qtr * 8 + sl
                        vps = psp.tile([128, DV], F32, tag="projps")
                        for m in range(NMC):
                            nc.tensor.matmul(
                                vps[:], tin[:, m, sl * 128:(sl + 1) * 128], wv_sb[:, m, :],
                                start=(m == 0), stop=(m == NMC - 1),
                            )
                        if collective:
                            nc.vector.tensor_copy(V_loc[:, kt_idx, :], vps[:])
                        else:
                            nc.vector.tensor_copy(VA[:, kt_idx, 0:DV], vps[:])

            if collective:
                cc_in_v = dramp.tile([128, SK], BF16)
                cc_out_v = dramp.tile([2, 128, SK], BF16)
                nc.sync.dma_start(cc_in_v.rearrange("p (kt d) -> p kt d", d=DV), V_loc[:])
                nc.gpsimd.collective_compute(
                    "AllGather",
                    mybir.AluOpType.bypass,
                    replica_groups=[[0, 1], [2, 3], [4, 5], [6, 7]],
                    ins=[cc_in_v[:]],
                    outs=[cc_out_v[:]],
                )

            # ---- Q projection (collective mode: after V so collectives overlap) ----
            if collective:
                for qtr, tin in proj_quarters(qT, SQ):
                    for half in range(2):
                        cb = qtr * 2 + half
                        proj512(wq_sb, tin, half, QT[:, cb * 512:(cb + 1) * 512])

            # ---- masks ----
            mska_sb = constp.tile([128, 128], BF16)
            nc.sync.dma_start(mska_sb[:], mska[:])
            mskb_sb = constp.tile([128, 128], BF16)
            nc.sync.dma_start(mskb_sb[:], mskb[:])
            zbias = constp.tile([128, 1], F32)
            nc.vector.memset(zbias[:], 0.0)

            # ---- unpack gathered K/V ----
            if collective:
                for r in range(2):
                    nc.sync.dma_start(
                        KT[:, r * SK:(r + 1) * SK], cc_out_k[r]
                    )
                    nc.sync.dma_start(
                        VA[:, r * nloc_kt:(r + 1) * nloc_kt, 0:DV],
                        cc_out_v[r].rearrange("p (kt d) -> p kt d", d=DV),
                    )

            # ---- attention, per q-chunk of 512 ----
            for cc in range(NCH):
                npair = 4 * cc + 4
                PT = ptp.tile([128, MAXKT, 512], BF16, tag="pt")
                for a in range(npair):
                    j0 = max(0, a - 4 * cc)
                    n = (4 - j0) * 128
                    qoff = cc * 512 + j0 * 128
                    st = pstp.tile([128, 2, 512], F32, tag="stps")
                    for half in range(2):
                        kt = 2 * a + half
                        nc.tensor.matmul(
                            st[:, half, :n],
                            KT[:, kt * 128:(kt + 1) * 128],
                            QT[:, qoff:qoff + n],
                            start=True, stop=True,
                        )
                    nc.scalar.activation(
                        PT[:, 2 * a:2 * a + 2, j0 * 128:512],
                        st[:, :, :n],
                        Exp, bias=zbias[:],
                    )
                    for j in range(j0, 4):
                        i = 4 * cc + j
                        for half in range(2):
                            kt = 2 * a + half
                            msk = None
                            if kt == 2 * i:
                                msk = mska_sb
                            elif kt == 2 * i + 1:
                                msk = mskb_sb
                            if msk is not None:
                                sl = PT[:, kt, j * 128:(j + 1) * 128]
                                nc.vector.tensor_mul(sl, sl, msk[:])

                for j in range(4):
                    i = 4 * cc + j
                    nkt_i = 2 * i + 2
                    po = psop.tile([128, DV + 1], F32, tag="ops")
                    for kt in range(nkt_i):
                        nc.tensor.matmul(
                            po[:], PT[:, kt, j * 128:(j + 1) * 128], VA[:, kt, 0:DV + 1],
                            start=(kt == 0), stop=(kt == nkt_i - 1),
                        )
                    rec = outp.tile([128, 1], F32, tag="rec")
                    nc.vector.reciprocal(rec[:], po[:, DV:DV + 1])
                    ob = outp.tile([128, DV], F32, tag="ob")
                    nc.vector.tensor_scalar_mul(ob[:], po[:, 0:DV], rec[:])
                    nc.sync.dma_start(out[i * 128:(i + 1) * 128, :], ob[:])

    nc.compile()
    return nc


def build_nc_v3():
    """v3: keys split by tile parity within each batch pair; K^T and V are
    exchanged with *segmented* AllGathers (K: 2 segs, V: 3 segs) so attention
    score chunks start as soon as their key range has landed.  Attention is
    interleaved into the projection stream: the PE instruction order follows
    expected data-arrival order so no engine waits on late data.

    Layouts: KT4 [128(dk), r, j, 128] with global key tile g = 2j + r
    (r = producing rank parity), VA [128(key), r, j, DV+2] with ones column
    at [:, :, :, DV] for the softmax denominator."""
    NLT = SK // 128            # 16 local key tiles per core
    RG = [[0, 1], [2, 3], [4, 5], [6, 7]]

    nc = bacc.Bacc(None, target_bir_lowering=False, num_devices=8)

    qT = nc.declare_dram_parameter("qT", [DM, SQ], BF16, isOutput=False)
    kT = nc.declare_dram_parameter("kT", [DM, SK], BF16, isOutput=False)
    vT = nc.declare_dram_parameter("vT", [DM, SK], BF16, isOutput=False)
    wq = nc.declare_dram_parameter("wq", [128, NMC * DK], BF16, isOutput=False)
    wk = nc.declare_dram_parameter("wk", [128, NMC * DK], BF16, isOutput=False)
    wv = nc.declare_dram_parameter("wv", [128, NMC * DV], BF16, isOutput=False)
    mska = nc.declare_dram_parameter("mska", [128, 128], BF16, isOutput=False)
    mskb = nc.declare_dram_parameter("mskb", [128, 128], BF16, isOutput=False)
    out = nc.declare_dram_parameter("out", [SQ, DV], F32, isOutput=True)

    Exp = mybir.ActivationFunctionType.Exp

    with tile.TileContext(nc) as tc:
        with (
            tc.tile_pool(name="const", bufs=1) as constp,
            tc.tile_pool(name="stream", bufs=3) as streamp,
            tc.tile_pool(name="big", bufs=1) as bigp,
            tc.tile_pool(name="ptp", bufs=2) as ptp,
            tc.tile_pool(name="outp", bufs=4) as outp,
            tc.tile_pool(name="dram", bufs=1, space="DRAM") as dramp,
            tc.tile_pool(name="ps", bufs=2, space="PSUM") as psp,
            tc.tile_pool(name="pst", bufs=2, space="PSUM") as pstp,
            tc.tile_pool(name="pso", bufs=2, space="PSUM") as psop,
        ):
            # ---- constants (wk first: K projection starts earliest) ----
            wk_sb = constp.tile([128, NMC, DK], BF16)
            nc.sync.dma_start(wk_sb[:], wk.rearrange("p (mc d) -> p mc d", d=DK))
            mska_sb = constp.tile([128, 128], BF16)
            nc.sync.dma_start(mska_sb[:], mska[:])
            mskb_sb = constp.tile([128, 128], BF16)
            nc.sync.dma_start(mskb_sb[:], mskb[:])
            wq_sb = constp.tile([128, NMC, DK], BF16)
            nc.sync.dma_start(wq_sb[:], wq.rearrange("p (mc d) -> p mc d", d=DK))
            wv_sb = constp.tile([128, NMC, DV], BF16)
            nc.sync.dma_start(wv_sb[:], wv.rearrange("p (mc d) -> p mc d", d=DV))

            zbias = constp.tile([128, 1], F32)
            nc.vector.memset(zbias[:], 0.0)

            # ---- persistent activations ----
            QT = bigp.tile([128, SQ], BF16)
            KT_loc = bigp.tile([128, SK], BF16)
            V_loc = bigp.tile([128, NLT, DV], BF16)
            KT4 = bigp.tile([128, 2, NLT, 128], BF16)
            VA = bigp.tile([128, 2, NLT, DV + 2], BF16)
            nc.vector.memset(VA[:], 1.0)  # ones at [:, :, :, DV]

            # ---- PE warm-up (HAM to 8/8 before first projection) ----
            warm = constp.tile([128, 256], BF16)
            nc.vector.memset(warm[:], 0.0)
            wps = psp.tile([128, 256], F32, tag="projps")
            for _ in range(20):
                nc.tensor.matmul(wps[:], warm[:, 0:128], warm[:], start=True, stop=True)

            # ---- collective buffers ----
            KSEG = [(0, 8), (8, 8)]            # (j0, nj) local-tile ranges
            VSEG = [(0, 8), (8, 4), (12, 4)]
            cc_in_k = [dramp.tile([128, nj * 128], BF16, name=f"cik{s}")
                       for s, (j0, nj) in enumerate(KSEG)]
            cc_out_k = [dramp.tile([2, 128, nj * 128], BF16, name=f"cok{s}")
                        for s, (j0, nj) in enumerate(KSEG)]
            cc_in_v = [dramp.tile([128, nj * DV], BF16, name=f"civ{s}")
                       for s, (j0, nj) in enumerate(VSEG)]
            cc_out_v = [dramp.tile([2, 128, nj * DV], BF16, name=f"cov{s}")
                        for s, (j0, nj) in enumerate(VSEG)]

            def ag_k(s):
                j0, nj = KSEG[s]
                nc.sync.dma_start(cc_in_k[s][:], KT_loc[:, j0 * 128:(j0 + nj) * 128])
                nc.gpsimd.collective_compute(
                    "AllGather", mybir.AluOpType.bypass, replica_groups=RG,
                    ins=[cc_in_k[s][:]], outs=[cc_out_k[s][:]],
                )
                for r in range(2):
                    nc.sync.dma_start(
                        KT4[:, r, j0:j0 + nj, :],
                        cc_out_k[s][r].rearrange("p (j c) -> p j c", c=128),
                    )

            def ag_v(s):
                j0, nj = VSEG[s]
                nc.sync.dma_start(cc_in_v[s][:], V_loc[:, j0:j0 + nj, :])
                nc.gpsimd.collective_compute(
                    "AllGather", mybir.AluOpType.bypass, replica_groups=RG,
                    ins=[cc_in_v[s][:]], outs=[cc_out_v[s][:]],
                )
                for r in range(2):
                    nc.sync.dma_start(
                        VA[:, r, j0:j0 + nj, 0:DV],
                        cc_out_v[s][r].rearrange("p (j d) -> p j d", d=DV),
                    )

            # ---- streamed projections (512-col chunks) ----
            def stream_chunk(src_dram, c):
                tin = streamp.tile([128, NMC, 512], BF16, tag="instream")
                nc.sync.dma_start(
                    tin[:],
                    src_dram[:, c * 512:(c + 1) * 512].rearrange(
                        "(mc p) c -> p mc c", p=128
                    ),
                )
                return tin

            def proj512(w_sb, tin, dst_sb_slice):
                ps = psp.tile([128, 512], F32, tag="projps")
                for m in range(NMC):
                    nc.tensor.matmul(
                        ps[:], w_sb[:, m, :], tin[:, m, :],
                        start=(m == 0), stop=(m == NMC - 1),
                    )
                nc.vector.tensor_copy(dst_sb_slice, ps[:])

            def vproj_chunk(tin, c):
                for sl in range(4):
                    lt = c * 4 + sl
                    vps = psp.tile([128, DV], F32, tag="projps")
                    for m in range(NMC):
                        nc.tensor.matmul(
                            vps[:], tin[:, m, sl * 128:(sl + 1) * 128], wv_sb[:, m, :],
                            start=(m == 0), stop=(m == NMC - 1),
                        )
                    nc.vector.tensor_copy(V_loc[:, lt, :], vps[:])

            # ---- attention pieces ----
            PT_tiles = {}

            def scores_chunk(cc):
                npair = 4 * cc + 4
                PT = ptp.tile([128, MAXKT, 512], BF16, tag="pt")
                PT_tiles[cc] = PT
                for a in range(npair):
                    j0 = max(0, a - 4 * cc)
                    n = (4 - j0) * 128
                    qoff = cc * 512 + j0 * 128
                    st = pstp.tile([128, 2, 512], F32, tag="stps")
                    for half in range(2):
                        kt = 2 * a + half
                        nc.tensor.matmul(
                            st[:, half, :n],
                            KT4[:, kt % 2, kt // 2, :],
                            QT[:, qoff:qoff + n],
                            start=True, stop=True,
                        )
                    nc.scalar.activation(
                        PT[:, 2 * a:2 * a + 2, j0 * 128:512],
                        st[:, :, :n],
                        Exp, bias=zbias[:],
                    )
                    for j in range(j0, 4):
                        i = 4 * cc + j
                        for half in range(2):
                            kt = 2 * a + half
                            msk = None
                            if kt == 2 * i:
                                msk = mska_sb
                            elif kt == 2 * i + 1:
                                msk = mskb_sb
                            if msk is not None:
                                sl = PT[:, kt, j * 128:(j + 1) * 128]
                                nc.vector.tensor_mul(sl, sl, msk[:])

            def pv_chunk(cc):
                PT = PT_tiles[cc]
                for j in range(4):
                    i = 4 * cc + j
                    nkt_i = 2 * i + 2
                    po = psop.tile([128, DV + 1], F32, tag="ops")
                    for kt in range(nkt_i):
                        nc.tensor.matmul(
                            po[:], PT[:, kt, j * 128:(j + 1) * 128],
                            VA[:, kt % 2, kt // 2, 0:DV + 1],
                            start=(kt == 0), stop=(kt == nkt_i - 1),
                        )
                    rec = outp.tile([128, 1], F32, tag="rec")
                    nc.vector.reciprocal(rec[:], po[:, DV:DV + 1])
                    ob = outp.tile([128, DV], F32, tag="ob")
                    nc.vector.tensor_scalar_mul(ob[:], po[:, 0:DV], rec[:])
                    nc.sync.dma_start(out[i * 128:(i + 1) * 128, :], ob[:])

            # ---- the interleaved schedule ----
            # arrival/PE order: k0 q0 k1 q1 | sc0 sc1 | k2 k3 q2 sc2 |
            #                   v0 v1 q3 sc3 | v2 v3 | pv0..pv3
            def kproj(c):
                tin = stream_chunk(kT, c)
                proj512(wk_sb, tin, KT_loc[:, c * 512:(c + 1) * 512])

            def qproj(c):
                tin = stream_chunk(qT, c)
                proj512(wq_sb, tin, QT[:, c * 512:(c + 1) * 512])

            kproj(0)
            qproj(0)
            kproj(1)
            ag_k(0)
            qproj(1)
            scores_chunk(0)
            scores_chunk(1)
            kproj(2)
            kproj(3)
            ag_k(1)
            qproj(2)
            vproj_chunk(stream_chunk(vT, 0), 0)
            scores_chunk(2)
            vproj_chunk(stream_chunk(vT, 1), 1)
            ag_v(0)
            qproj(3)
            scores_chunk(3)
            vproj_chunk(stream_chunk(vT, 2), 2)
            ag_v(1)
            vproj_chunk(stream_chunk(vT, 3), 3)
            ag_v(2)
            pv_chunk(0)
            pv_chunk(1)
            pv_chunk(2)
            pv_chunk(3)

    nc.compile()
    return nc


def build_nc_v4():
    """v4 = v3 with the trigger-queue serialization fixed:

    - all collective staging (SBUF->DRAM) and unpack (DRAM->SBUF) DMAs run on
      the GpSimd (SWDGE) queue, whose in-order semantics match their true
      dependencies, leaving the Sync queue a pure linear input/output stream;
    - 1024-col input chunks (2 KB DMA lines, half the trigger count);
    - weights and masks merged into single params (2 const DMAs);
    - outputs staged per 512-query chunk (4 output DMAs instead of 16);
    - V carries its denominator ones-columns through the AllGather so the
      unpack is a single contiguous DMA."""
    NLT = SK // 128            # 16 local key tiles per core
    DVP = DV + 2               # V row padded with ones at [DV] (and [DV+1])
    RG = [[0, 1], [2, 3], [4, 5], [6, 7]]

    nc = bacc.Bacc(None, target_bir_lowering=False, num_devices=8)

    qT = nc.declare_dram_parameter("qT", [DM, SQ], BF16, isOutput=False)
    kT = nc.declare_dram_parameter("kT", [DM, SK], BF16, isOutput=False)
    vT = nc.declare_dram_parameter("vT", [DM, SK], BF16, isOutput=False)
    w = nc.declare_dram_parameter("w", [128, 3 * NMC * DK], BF16, isOutput=False)
    msk = nc.declare_dram_parameter("msk", [128, 256], BF16, isOutput=False)
    out = nc.declare_dram_parameter("out", [SQ, DV], F32, isOutput=True)

    Exp = mybir.ActivationFunctionType.Exp

    with tile.TileContext(nc) as tc:
        with (
            tc.tile_pool(name="const", bufs=1) as constp,
            tc.tile_pool(name="stream", bufs=3) as streamp,
            tc.tile_pool(name="big", bufs=1) as bigp,
            tc.tile_pool(name="ptp", bufs=2) as ptp,
            tc.tile_pool(name="outp", bufs=4) as outp,
            tc.tile_pool(name="dram", bufs=1, space="DRAM") as dramp,
            tc.tile_pool(name="ps", bufs=2, space="PSUM") as psp,
            tc.tile_pool(name="pst", bufs=2, space="PSUM") as pstp,
            tc.tile_pool(name="pso", bufs=2, space="PSUM") as psop,
        ):
            # ---- consts ----
            w_sb = constp.tile([128, 3, NMC, DK], BF16)
            nc.sync.dma_start(
                w_sb[:], w.rearrange("p (t mc d) -> p t mc d", mc=NMC, d=DK)
            )
            msk_sb = constp.tile([128, 2, 128], BF16)
            nc.sync.dma_start(msk_sb[:], msk.rearrange("p (t c) -> p t c", c=128))
            zbias = constp.tile([128, 1], F32)
            nc.vector.memset(zbias[:], 0.0)

            # ---- persistent activations ----
            QT = bigp.tile([128, SQ], BF16)
            KT_loc = bigp.tile([128, SK], BF16)
            V_loc = bigp.tile([128, NLT, DVP], BF16)
            nc.vector.memset(V_loc[:], 1.0)  # ones at [:, :, DV:]
            KT4 = bigp.tile([128, 2, NLT, 128], BF16)
            VA = bigp.tile([128, 2, NLT, DVP], BF16)

            # ---- PE warm-up ----
            warm = constp.tile([128, 256], BF16)
            nc.vector.memset(warm[:], 0.0)
            wps = psp.tile([128, 256], F32, tag="projps")
            for _ in range(30):
                nc.tensor.matmul(wps[:], warm[:, 0:128], warm[:], start=True, stop=True)

            # ---- collective buffers (segments over local-tile index j) ----
            KSEG = [(0, 8), (8, 8)]
            VSEG = [(0, 8), (8, 4), (12, 4)]
            cc_in_k = [dramp.tile([128, nj * 128], BF16, name=f"cik{s}")
                       for s, (j0, nj) in enumerate(KSEG)]
            cc_out_k = [dramp.tile([2, 128, nj * 128], BF16, name=f"cok{s}")
                        for s, (j0, nj) in enumerate(KSEG)]
            cc_in_v = [dramp.tile([128, nj * DVP], BF16, name=f"civ{s}")
                       for s, (j0, nj) in enumerate(VSEG)]
            cc_out_v = [dramp.tile([2, 128, nj * DVP], BF16, name=f"cov{s}")
                        for s, (j0, nj) in enumerate(VSEG)]

            # Queue map (each engine's stream ordered to match true deps):
            #   scalar: cik/upk interleaved with the exps that consume KT4
            #   gpsimd: AllGathers + civ staging (idle otherwise)
            #   sync:   input/output streams + upv (free after inputs drain)
            def ag_k(s):
                j0, nj = KSEG[s]
                nc.scalar.dma_start(cc_in_k[s][:], KT_loc[:, j0 * 128:(j0 + nj) * 128])
                nc.gpsimd.collective_compute(
                    "AllGather", mybir.AluOpType.bypass, replica_groups=RG,
                    ins=[cc_in_k[s][:]], outs=[cc_out_k[s][:]],
                )
                nc.scalar.dma_start(
                    KT4[:, :, j0:j0 + nj, :],
                    cc_out_k[s].rearrange("r p (j c) -> p r j c", c=128),
                )

            def ag_v(s):
                j0, nj = VSEG[s]
                nc.gpsimd.dma_start(cc_in_v[s][:], V_loc[:, j0:j0 + nj, :])
                nc.gpsimd.collective_compute(
                    "AllGather", mybir.AluOpType.bypass, replica_groups=RG,
                    ins=[cc_in_v[s][:]], outs=[cc_out_v[s][:]],
                )
                nc.sync.dma_start(
                    VA[:, :, j0:j0 + nj, :],
                    cc_out_v[s].rearrange("r p (j d) -> p r j d", d=DVP),
                )

            # ---- streamed projections (1024-col chunks) ----
            def stream_chunk(src_dram, c):
                tin = streamp.tile([128, NMC, 1024], BF16, tag="instream")
                nc.sync.dma_start(
                    tin[:],
                    src_dram[:, c * 1024:(c + 1) * 1024].rearrange(
                        "(mc p) c -> p mc c", p=128
                    ),
                )
                return tin

            def proj512(wi, tin, half, dst_sb_slice):
                ps = psp.tile([128, 512], F32, tag="projps")
                for m in range(NMC):
                    nc.tensor.matmul(
                        ps[:], w_sb[:, wi, m, :], tin[:, m, half * 512:(half + 1) * 512],
                        start=(m == 0), stop=(m == NMC - 1),
                    )
                nc.vector.tensor_copy(dst_sb_slice, ps[:])

            def kproj(c):  # c in 0..1, covers local tiles 8c..8c+7
                tin = stream_chunk(kT, c)
                for half in range(2):
                    cb = 2 * c + half
                    proj512(1, tin, half, KT_loc[:, cb * 512:(cb + 1) * 512])

            def qproj(c):  # c in 0..1, covers q chunks 2c..2c+1
                tin = stream_chunk(qT, c)
                for half in range(2):
                    cb = 2 * c + half
                    proj512(0, tin, half, QT[:, cb * 512:(cb + 1) * 512])

            def vproj4(tin, c, half):  # 4 local tiles c*8+half*4 ..+3
                for sl in range(4 * half, 4 * half + 4):
                    lt = c * 8 + sl
                    vps = psp.tile([128, DV], F32, tag="projps")
                    for m in range(NMC):
                        nc.tensor.matmul(
                            vps[:], tin[:, m, sl * 128:(sl + 1) * 128],
                            w_sb[:, 2, m, :],
                            start=(m == 0), stop=(m == NMC - 1),
                        )
                    nc.vector.tensor_copy(V_loc[:, lt, 0:DV], vps[:])

            # ---- attention ----
            PT_tiles = {}

            def scores_chunk(cc):
                npair = 4 * cc + 4
                PT = ptp.tile([128, MAXKT, 512], BF16, tag="pt")
                PT_tiles[cc] = PT
                for a in range(npair):
                    j0 = max(0, a - 4 * cc)
                    n = (4 - j0) * 128
                    qoff = cc * 512 + j0 * 128
                    st = pstp.tile([128, 2, 512], F32, tag="stps")
                    for half in range(2):
                        kt = 2 * a + half
                        nc.tensor.matmul(
                            st[:, half, :n],
                            KT4[:, kt % 2, kt // 2, :],
                            QT[:, qoff:qoff + n],
                            start=True, stop=True,
                        )
                    nc.scalar.activation(
                        PT[:, 2 * a:2 * a + 2, j0 * 128:512],
                        st[:, :, :n],
                        Exp, bias=zbias[:],
                    )
                    for j in range(j0, 4):
                        i = 4 * cc + j
                        for half in range(2):
                            kt = 2 * a + half
                            mi = None
                            if kt == 2 * i:
                                mi = 0
                            elif kt == 2 * i + 1:
                                mi = 1
                            if mi is not None:
                                sl = PT[:, kt, j * 128:(j + 1) * 128]
                                nc.vector.tensor_mul(sl, sl, msk_sb[:, mi, :])

            def pv_chunk(cc):
                PT = PT_tiles[cc]
                ostg = outp.tile([128, 4, DV], F32, tag="ostg")
                for j in range(4):
                    i = 4 * cc + j
                    nkt_i = 2 * i + 2
                    po = psop.tile([128, DV + 1], F32, tag="ops")
                    for kt in range(nkt_i):
                        nc.tensor.matmul(
                            po[:], PT[:, kt, j * 128:(j + 1) * 128],
                            VA[:, kt % 2, kt // 2, 0:DV + 1],
                            start=(kt == 0), stop=(kt == nkt_i - 1),
                        )
                    rec = outp.tile([128, 1], F32, tag="rec")
                    nc.vector.reciprocal(rec[:], po[:, DV:DV + 1])
                    nc.vector.tensor_scalar_mul(ostg[:, j, :], po[:, 0:DV], rec[:])
                nc.sync.dma_start(
                    out[cc * 512:(cc + 1) * 512, :].rearrange("(t p) d -> p t d", p=128),
                    ostg[:],
                )

            # ---- schedule ----
            kproj(0)
            ag_k(0)
            qproj(0)
            scores_chunk(0)
            scores_chunk(1)
            kproj(1)
            ag_k(1)
            qproj(1)
            scores_chunk(2)
            scores_chunk(3)
            tin_v0 = stream_chunk(vT, 0)
            vproj4(tin_v0, 0, 0)
            vproj4(tin_v0, 0, 1)
            ag_v(0)
            tin_v1 = stream_chunk(vT, 1)
            vproj4(tin_v1, 1, 0)
            ag_v(1)
            vproj4(tin_v1, 1, 1)
            ag_v(2)
            pv_chunk(0)
            pv_chunk(1)
            pv_chunk(2)
            pv_chunk(3)

    nc.compile()
    return nc


def build_nc_v6():
    """v6: no collectives (measured: cc DMAs starve ~20x against the bulk
    input stream, and DRAM round-trips cost as much as the duplicated
    projections they replace).  Each core projects its batch's full K/V;
    attention chunks are interleaved into the projection stream so the PE
    never phase-waits.  Stream order feeds the exp (ScalarE) path first:
    k0 q0 k1 q1 k2 k3 v0..v3."""
    RG = None  # no collectives
    DVP = DV + 2

    nc = bacc.Bacc(None, target_bir_lowering=False, num_devices=8)

    qT = nc.declare_dram_parameter("qT", [DM, SQ], BF16, isOutput=False)
    kT = nc.declare_dram_parameter("kT", [DM, S], BF16, isOutput=False)
    vT = nc.declare_dram_parameter("vT", [DM, S], BF16, isOutput=False)
    wk = nc.declare_dram_parameter("wk", [128, NMC * DK], BF16, isOutput=False)
    wqv = nc.declare_dram_parameter("wqv", [128, 2 * NMC * DK], BF16, isOutput=False)
    msk = nc.declare_dram_parameter("msk", [128, 256], BF16, isOutput=False)
    out = nc.declare_dram_parameter("out", [SQ, DV], F32, isOutput=True)

    Exp = mybir.ActivationFunctionType.Exp

    with tile.TileContext(nc) as tc:
        with (
            tc.tile_pool(name="const", bufs=1) as constp,
            tc.tile_pool(name="stream", bufs=5) as streamp,
            tc.tile_pool(name="big", bufs=1) as bigp,
            tc.tile_pool(name="ptp", bufs=2) as ptp,
            tc.tile_pool(name="outp", bufs=4) as outp,
            tc.tile_pool(name="ps", bufs=2, space="PSUM") as psp,
            tc.tile_pool(name="pst", bufs=2, space="PSUM") as pstp,
            tc.tile_pool(name="pso", bufs=2, space="PSUM") as psop,
        ):
            # wk first so the first K projection is gated only by wk + k0a
            wk_sb = constp.tile([128, NMC, DK], BF16)
            nc.sync.dma_start(wk_sb[:], wk.rearrange("p (mc d) -> p mc d", d=DK))
            msk_sb = constp.tile([128, 2, 128], BF16)
            nc.sync.dma_start(msk_sb[:], msk.rearrange("p (t c) -> p t c", c=128))

            # warm-up tile memset FIRST on DVE so the PE warm-up isn't gated
            # behind the big VA fill
            warm = constp.tile([128, 256], BF16)
            nc.vector.memset(warm[:], 0.0)
            wps = psp.tile([128, 256], F32, tag="projps")
            for _ in range(12):
                nc.tensor.matmul(wps[:], warm[:, 0:128], warm[:], start=True, stop=True)

            zbias = constp.tile([128, 1], F32)
            nc.vector.memset(zbias[:], 0.0)

            QT = bigp.tile([128, SQ], BF16)
            KT = bigp.tile([128, S], BF16)
            VA = bigp.tile([128, MAXKT, DVP], BF16)
            nc.vector.memset(VA[:], 1.0)

            def stream_chunk(src_dram, c0, ncols):
                nmcv = ncols  # free-dim cols of this chunk
                tin = streamp.tile([128, NMC, nmcv], BF16, tag="instream",
                                   name=f"in{c0}_{ncols}")
                nc.sync.dma_start(
                    tin[:],
                    src_dram[:, c0:c0 + ncols].rearrange("(mc p) c -> p mc c", p=128),
                )
                return tin

            wqv_sb = constp.tile([128, 2, NMC, DK], BF16)

            def proj512(wt, tin, half, dst_sb_slice):
                ps = psp.tile([128, 512], F32, tag="projps")
                for m in range(NMC):
                    nc.tensor.matmul(
                        ps[:], wt[:, m, :], tin[:, m, half * 512:(half + 1) * 512],
                        start=(m == 0), stop=(m == NMC - 1),
                    )
                nc.vector.tensor_copy(dst_sb_slice, ps[:])

            def kproj(c0, ncols):  # projects key cols [c0, c0+ncols)
                tin = stream_chunk(kT, c0, ncols)
                for half in range(ncols // 512):
                    cb = c0 // 512 + half
                    proj512(wk_sb, tin, half, KT[:, cb * 512:(cb + 1) * 512])

            def qproj(c0, ncols):
                tin = stream_chunk(qT, c0, ncols)
                for half in range(ncols // 512):
                    cb = c0 // 512 + half
                    proj512(wqv_sb[:, 0], tin, half, QT[:, cb * 512:(cb + 1) * 512])

            def vproj(c0, ncols):  # key cols [c0, c0+ncols) -> tiles c0//128..
                tin = stream_chunk(vT, c0, ncols)
                for sl in range(ncols // 128):
                    lt = c0 // 128 + sl
                    vps = psp.tile([128, DV], F32, tag="projps")
                    for m in range(NMC):
                        nc.tensor.matmul(
                            vps[:], tin[:, m, sl * 128:(sl + 1) * 128],
                            wqv_sb[:, 1, m, :],
                            start=(m == 0), stop=(m == NMC - 1),
                        )
                    nc.vector.tensor_copy(VA[:, lt, 0:DV], vps[:])

            PT_tiles = {}

            def scores_chunk(cc):
                npair = 4 * cc + 4
                PT = ptp.tile([128, MAXKT, 512], BF16, tag="pt")
                PT_tiles[cc] = PT
                for a in range(npair):
                    j0 = max(0, a - 4 * cc)
                    n = (4 - j0) * 128
                    qoff = cc * 512 + j0 * 128
                    st = pstp.tile([128, 2, 512], F32, tag="stps")
                    for half in range(2):
                        kt = 2 * a + half
                        nc.tensor.matmul(
                            st[:, half, :n],
                            KT[:, kt * 128:(kt + 1) * 128],
                            QT[:, qoff:qoff + n],
                            start=True, stop=True,
                        )
                    nc.scalar.activation(
                        PT[:, 2 * a:2 * a + 2, j0 * 128:512],
                        st[:, :, :n],
                        Exp, bias=zbias[:],
                    )
                    for j in range(j0, 4):
                        i = 4 * cc + j
                        for half in range(2):
                            kt = 2 * a + half
                            mi = 0 if kt == 2 * i else (1 if kt == 2 * i + 1 else None)
                            if mi is not None:
                                sl = PT[:, kt, j * 128:(j + 1) * 128]
                                nc.vector.tensor_mul(sl, sl, msk_sb[:, mi, :])

            def pv_chunk(cc):
                PT = PT_tiles[cc]
                ostg = outp.tile([128, 4, DV], F32, tag="ostg")
                for j in range(4):
                    i = 4 * cc + j
                    nkt_i = 2 * i + 2
                    po = psop.tile([128, DV + 1], F32, tag="ops")
                    for kt in range(nkt_i):
                        nc.tensor.matmul(
                            po[:], PT[:, kt, j * 128:(j + 1) * 128],
                            VA[:, kt, 0:DV + 1],
                            start=(kt == 0), stop=(kt == nkt_i - 1),
                        )
                    rec = outp.tile([128, 1], F32, tag="rec")
                    nc.vector.reciprocal(rec[:], po[:, DV:DV + 1])
                    nc.vector.tensor_scalar_mul(ostg[:, j, :], po[:, 0:DV], rec[:])
                nc.sync.dma_start(
                    out[cc * 512:(cc + 1) * 512, :].rearrange("(t p) d -> p t d", p=128),
                    ostg[:],
                )

            def finish_row(po, ostg, j):
                rec = outp.tile([128, 1], F32, tag="rec")
                nc.vector.reciprocal(rec[:], po[:, DV:DV + 1])
                nc.vector.tensor_scalar_mul(ostg[:, j, :], po[:, 0:DV], rec[:])

            # schedule: stream order = wk msk k0a k0b wqv q0 k1 q1 k2 k3
            #                          v0 v1 v2 v3a v3b
            kproj(0, 512)
            kproj(512, 512)
            nc.sync.dma_start(
                wqv_sb[:], wqv.rearrange("p (t mc d) -> p t mc d", mc=NMC, d=DK)
            )
            qproj(0, 1024)
            scores_chunk(0)
            kproj(1024, 1024)
            scores_chunk(1)
            qproj(1024, 1024)
            kproj(2048, 1024)
            scores_chunk(2)
            kproj(3072, 1024)
            scores_chunk(3)
            vproj(0, 1024)
            vproj(1024, 1024)
            pv_chunk(0)
            pv_chunk(1)
            vproj(2048, 1024)
            pv_chunk(2)
            vproj(3072, 512)
            # pv3 split: accumulate kt<=27 while v tiles 28..31 stream in
            PT3 = PT_tiles[3]
            ostg3 = outp.tile([128, 4, DV], F32, tag="ostg")
            pos3 = []
            for j in range(4):
                i = 12 + j
                po = psop.tile([128, DV + 1], F32, tag="ops")
                pos3.append(po)
                last = min(2 * i + 1, 27)
                for kt in range(last + 1):
                    nc.tensor.matmul(
                        po[:], PT3[:, kt, j * 128:(j + 1) * 128],
                        VA[:, kt, 0:DV + 1],
                        start=(kt == 0), stop=(kt == 2 * i + 1),
                    )
                if 2 * i + 1 <= 27:
                    finish_row(po, ostg3, j)
            vproj(3584, 512)
            for j in (2, 3):
                i = 12 + j
                po = pos3[j]
                for kt in range(28, 2 * i + 2):
                    nc.tensor.matmul(
                        po[:], PT3[:, kt, j * 128:(j + 1) * 128],
                        VA[:, kt, 0:DV + 1],
                        start=False, stop=(kt == 2 * i + 1),
                    )
                finish_row(po, ostg3, j)
            nc.sync.dma_start(
                out[3 * 512:4 * 512, :].rearrange("(t p) d -> p t d", p=128),
                ostg3[:],
            )

    nc.compile()
    return nc


def _prep_inputs(q, k, v, W_Q, W_K, W_V, mode=None):
    mode = MODE if mode is None else mode
    collective = mode == "v2"
    q = np.asarray(q, dtype=np.float32)
    k = np.asarray(k, dtype=np.float32)
    v = np.asarray(v, dtype=np.float32)
    W_Q = np.asarray(W_Q, dtype=np.float32)
    W_K = np.asarray(W_K, dtype=np.float32)
    W_V = np.asarray(W_V, dtype=np.float32)

    scale = 1.0 / math.sqrt(DK)

    def wtile(w):  # [128, 1024] pre-tiled: row p, cols mc*128+d
        return np.ascontiguousarray(
            w.T.reshape(NMC, 128, w.shape[0]).transpose(1, 0, 2).reshape(128, NMC * w.shape[0])
        ).astype(BF16NP)

    wq_h = wtile(W_Q * scale)
    wk_h = wtile(W_K)
    wv_h = wtile(W_V)
    tri = np.triu(np.ones((128, 128), np.float32)).astype(BF16NP)
    ones = np.ones((128, 128), BF16NP)
    zeros = np.zeros((128, 128), BF16NP)

    in_maps = []
    for c in range(8):
        b, p = c // 2, c % 2
        idx = np.arange(NQT) * 2 + p
        qsel = q[b].reshape(MAXKT, 128, DM)[idx].reshape(SQ, DM)
        if mode == "v6":
            kslc = k[b]
            vslc = v[b]
        elif mode in ("v3", "v4"):
            kidx = np.arange(SK // 128) * 2 + p
            kslc = k[b].reshape(MAXKT, 128, DM)[kidx].reshape(SK, DM)
            vslc = v[b].reshape(MAXKT, 128, DM)[kidx].reshape(SK, DM)
        elif collective:
            kslc = k[b, p * SK:(p + 1) * SK, :]
            vslc = v[b, p * SK:(p + 1) * SK, :]
        else:
            kslc = k[b]
            vslc = v[b]
        im = {
            "qT": np.ascontiguousarray(qsel.T).astype(BF16NP),
            "kT": np.ascontiguousarray(kslc.T).astype(BF16NP),
            "vT": np.ascontiguousarray(vslc.T).astype(BF16NP),
        }
        mska_h = ones if p == 1 else tri
        mskb_h = tri if p == 1 else zeros
        if mode == "v6":
            im["wk"] = wk_h
            im["wqv"] = np.ascontiguousarray(np.concatenate([wq_h, wv_h], axis=1))
            im["msk"] = np.ascontiguousarray(
                np.concatenate([mska_h, mskb_h], axis=1)
            )
        elif mode == "v4":
            im["w"] = np.ascontiguousarray(
                np.concatenate([wq_h, wk_h, wv_h], axis=1)
            )
            im["msk"] = np.ascontiguousarray(
                np.concatenate([mska_h, mskb_h], axis=1)
            )
        else:
            im.update({
                "wq": wq_h, "wk": wk_h, "wv": wv_h,
                "mska": mska_h, "mskb": mskb_h,
            })
            if mode != "v3":
                im["eye"] = np.eye(128, dtype=np.float32).astype(BF16NP)
        in_maps.append(im)
    return in_maps


def kernel(q, k, v, W_Q, W_K, W_V):
    global LAST_RESULTS
    if MODE not in _NC_CACHE:
        if MODE == "v6":
            _NC_CACHE[MODE] = build_nc_v6()
        elif MODE == "v4":
            _NC_CACHE[MODE] = build_nc_v4()
        elif MODE == "v3":
            _NC_CACHE[MODE] = build_nc_v3()
        else:
            _NC_CACHE[MODE] = build_nc(MODE)
    nc = _NC_CACHE[MODE]

    in_maps = _prep_inputs(q, k, v, W_Q, W_K, W_V)
    res = run_bass_kernel_spmd(nc, in_maps, core_ids=list(range(8)))
    LAST_RESULTS = res

    out = np.empty((B, S, DV), np.float32)
    for c in range(8):
        b, p = c // 2, c % 2
        oc = res.results[c]["out"]
        out[b].reshape(MAXKT, 128, DV)[np.arange(NQT) * 2 + p] = (
            oc.reshape(NQT, 128, DV)
        )
    return out



# revision 39
# speedup vs baseline: 1.1355x; 1.1355x over previous
"""Causal single-head attention (B=4, S=4096, D_MODEL=1024, D_K=D_V=128)
distributed over 8 TRN2 NeuronCores.

Sharding: batch (4) x interleaved query-tile parity (2) = 8 cores.
Core c handles batch b=c//2, parity p=c%2; its local q-tile i (16 tiles of
128 queries) is global q-tile T = 2*i + p.  The causal workload per-core is
identical (sum over i of (2i+2) key-tiles), so one SPMD program serves all
cores; the parity difference lives in two host-supplied [128,128] masks.

Shipped MODE "v6" (99.5 us, vs 119.3 us phase-separated baseline): each
core projects its batch's full K/V locally (collective variants v2/v3/v4
measured slower: AllGather DRAM round-trips starve ~20x against the bulk
input stream on this DMA arbiter, and cost as much HBM traffic as the
duplicated projections they replace).  Attention score/exp/PV chunks are
interleaved into the projection stream in data-arrival order
(k0 q0 k1 q1 k2 k3 v0..v3) so the PE never phase-waits; a short 12-MM
warm-up issued before the first data lands keeps the PE HAM clock at 8/8
(cold-clock inflation cost ~18 us in earlier revisions); stream pool depth
5 keeps the input DMA at bandwidth pace instead of demand pace.

Per-core compute layout ("all transposed", no PE transposes):
  - projections produce QT [dk=128, q], KT [dk=128, keys] (N=512 matmuls)
    and V [keys, dv] (N=128 matmuls)
  - scores ST = [keys=128, q<=512] tiles, two key-tiles per PSUM tile so
    one ACT exp covers a pair (ACT ops pay (N+352)/1.2 ns)
  - softmax: exp on ScalarE (scores bounded ~|z|<3: no max subtraction),
    causal via binary mask multiplies on VectorE
  - PV: matmul(lhsT=P_tile [keys,128q], rhs=V_aug [keys,129]) -> output in
    natural [q, dv] layout with the softmax denominator in column 128;
    normalization = DVE reciprocal + tensor_scalar_mul.
"""

import math
import numpy as np
import ml_dtypes

import concourse.bass as bass
import concourse.mybir as mybir
from concourse import bacc, tile
from concourse.bass_utils import run_bass_kernel_spmd

BF16NP = ml_dtypes.bfloat16
F32 = mybir.dt.float32
BF16 = mybir.dt.bfloat16

B = 4
S = 4096
DM = 1024
DK = 128
DV = 128
SQ = 2048          # queries per core
NQT = 16           # local q-tiles of 128
NMC = DM // 128    # 8 contraction chunks for projections
MAXKT = S // 128   # 32 key tiles
NCH = SQ // 512    # 4 q-chunks of 512
SK = S // 2        # keys projected locally per core (v2)

MODE = "v6"        # v15: full local K/V | v2/v3/v4: collective variants | v6: local K/V, interleaved attention

LAST_RESULTS = None
_NC_CACHE = {}


def build_nc(mode="v2", vt=False):
    collective = mode == "v2"
    nkeys = SK if collective else S

    nc = bacc.Bacc(None, target_bir_lowering=False, num_devices=8)

    qT = nc.declare_dram_parameter("qT", [DM, SQ], BF16, isOutput=False)
    kT = nc.declare_dram_parameter("kT", [DM, nkeys], BF16, isOutput=False)
    vT = nc.declare_dram_parameter("vT", [DM, nkeys], BF16, isOutput=False)
    wq = nc.declare_dram_parameter("wq", [128, NMC * DK], BF16, isOutput=False)
    wk = nc.declare_dram_parameter("wk", [128, NMC * DK], BF16, isOutput=False)
    wv = nc.declare_dram_parameter("wv", [128, NMC * DV], BF16, isOutput=False)
    mska = nc.declare_dram_parameter("mska", [128, 128], BF16, isOutput=False)
    mskb = nc.declare_dram_parameter("mskb", [128, 128], BF16, isOutput=False)
    eye = nc.declare_dram_parameter("eye", [128, 128], BF16, isOutput=False)
    out = nc.declare_dram_parameter("out", [SQ, DV], F32, isOutput=True)

    Exp = mybir.ActivationFunctionType.Exp

    with tile.TileContext(nc) as tc:
        with (
            tc.tile_pool(name="const", bufs=1) as constp,
            tc.tile_pool(name="stream", bufs=3) as streamp,
            tc.tile_pool(name="big", bufs=1) as bigp,
            tc.tile_pool(name="ptp", bufs=2) as ptp,
            tc.tile_pool(name="outp", bufs=4) as outp,
            tc.tile_pool(name="dram", bufs=1, space="DRAM") as dramp,
            tc.tile_pool(name="ps", bufs=2, space="PSUM") as psp,
            tc.tile_pool(name="pst", bufs=2, space="PSUM") as pstp,
            tc.tile_pool(name="pso", bufs=2, space="PSUM") as psop,
        ):
            # ---- weights (host pre-tiled to [128, mc*128] contiguous) ----
            wk_sb = constp.tile([128, NMC, DK], BF16)
            nc.sync.dma_start(wk_sb[:], wk.rearrange("p (mc d) -> p mc d", d=DK))
            wv_sb = constp.tile([128, NMC, DV], BF16)
            nc.sync.dma_start(wv_sb[:], wv.rearrange("p (mc d) -> p mc d", d=DV))
            wq_sb = constp.tile([128, NMC, DK], BF16)
            nc.sync.dma_start(wq_sb[:], wq.rearrange("p (mc d) -> p mc d", d=DK))

            # ---- PE warm-up: keep TensorE busy during the input-DMA lead-in
            # so HAM is at 8/8 when the first real matmul's data lands ----
            warm = constp.tile([128, 256], BF16)
            nc.vector.memset(warm[:], 0.0)
            wps = psp.tile([128, 256], F32, tag="projps")
            for _ in range(50):
                nc.tensor.matmul(wps[:], warm[:, 0:128], warm[:], start=True, stop=True)

            # ---- persistent activations ----
            QT = bigp.tile([128, SQ], BF16)
            KT = bigp.tile([128, S], BF16)
            VA = bigp.tile([128, MAXKT, DV + 2], BF16)

            # ---- projection helper: streams src in quarters of 1024 cols ----
            def proj_quarters(src_dram, n_cols):
                for qtr in range(n_cols // 1024):
                    tin = streamp.tile([128, NMC, 1024], BF16, tag="instream")
                    nc.sync.dma_start(
                        tin[:],
                        src_dram[:, qtr * 1024:(qtr + 1) * 1024].rearrange(
                            "(mc p) c -> p mc c", p=128
                        ),
                    )
                    yield qtr, tin

            def proj512(w_sb, tin, half, dst_sb_slice):
                ps = psp.tile([128, 512], F32, tag="projps")
                for m in range(NMC):
                    nc.tensor.matmul(
                        ps[:], w_sb[:, m, :], tin[:, m, half * 512:(half + 1) * 512],
                        start=(m == 0), stop=(m == NMC - 1),
                    )
                nc.vector.tensor_copy(dst_sb_slice, ps[:])

            # ---- Q projection first (non-collective): measured best PE order.
            # First two chunks are 512 cols so the very first matmul's data
            # arrives sooner. ----
            if not collective:
                cb = 0
                for w in (512, 512, 1024):
                    tin = streamp.tile([128, NMC, w], BF16, tag="instream",
                                       name=f"qin{cb}")
                    nc.sync.dma_start(
                        tin[:],
                        qT[:, cb * 512:cb * 512 + w].rearrange(
                            "(mc p) c -> p mc c", p=128
                        ),
                    )
                    for half in range(w // 512):
                        proj512(wq_sb, tin, half, QT[:, (cb + half) * 512:(cb + half + 1) * 512])
                    cb += w // 512

            # ---- K projection (local keys) ----
            if collective:
                KT_loc = bigp.tile([128, SK], BF16)
                k_dst = KT_loc
            else:
                k_dst = KT
            for qtr, tin in proj_quarters(kT, nkeys):
                for half in range(2):
                    cb = qtr * 2 + half
                    proj512(wk_sb, tin, half, k_dst[:, cb * 512:(cb + 1) * 512])

            if collective:
                cc_in_k = dramp.tile([128, SK], BF16)
                cc_out_k = dramp.tile([2, 128, SK], BF16)
                nc.sync.dma_start(cc_in_k[:], KT_loc[:])
                nc.gpsimd.collective_compute(
                    "AllGather",
                    mybir.AluOpType.bypass,
                    replica_groups=[[0, 1], [2, 3], [4, 5], [6, 7]],
                    ins=[cc_in_k[:]],
                    outs=[cc_out_k[:]],
                )

            # ---- V projection (local keys, natural [keys, dv] layout) ----
            nc.vector.memset(VA[:], 1.0)  # ones column at [:, :, DV]
            nloc_kt = nkeys // 128
            if collective:
                V_loc = bigp.tile([128, nloc_kt, DV], BF16)
            if vt and not collective:
                # N=512 VT projection + PE-mode transpose into VA tiles,
                # interleaved per 512-key chunk so HAM stays warm
                eye_sb = constp.tile([128, 128], BF16)
                nc.sync.dma_start(eye_sb[:], eye[:])
                VTS = bigp.tile([128, S], BF16)
                for qtr, tin in proj_quarters(vT, nkeys):
                    for half in range(2):
                        cb = qtr * 2 + half
                        proj512(wv_sb, tin, half, VTS[:, cb * 512:(cb + 1) * 512])
                        for sl in range(4):
                            kt_idx = cb * 4 + sl
                            tp = psop.tile([128, 128], BF16, tag="ops")
                            nc.tensor.transpose(
                                tp[:], VTS[:, kt_idx * 128:(kt_idx + 1) * 128], eye_sb[:]
                            )
                            nc.vector.tensor_copy(VA[:, kt_idx, 0:DV], tp[:])
            else:
                for qtr, tin in proj_quarters(vT, nkeys):
                    for sl in range(8):
                        kt_idx = qtr * 8 + sl
                        vps = psp.tile([128, DV], F32, tag="projps")
                        for m in range(NMC):
                            nc.tensor.matmul(
                                vps[:], tin[:, m, sl * 128:(sl + 1) * 128], wv_sb[:, m, :],
                                start=(m == 0), stop=(m == NMC - 1),
                            )
                        if collective:
                            nc.vector.tensor_copy(V_loc[:, kt_idx, :], vps[:])
                        else:
                            nc.vector.tensor_copy(VA[:, kt_idx, 0:DV], vps[:])

            if collective:
                cc_in_v = dramp.tile([128, SK], BF16)
                cc_out_v = dramp.tile([2, 128, SK], BF16)
                nc.sync.dma_start(cc_in_v.rearrange("p (kt d) -> p kt d", d=DV), V_loc[:])
                nc.gpsimd.collective_compute(
                    "AllGather",
                    mybir.AluOpType.bypass,
                    replica_groups=[[0, 1], [2, 3], [4, 5], [6, 7]],
                    ins=[cc_in_v[:]],
                    outs=[cc_out_v[:]],
                )

            # ---- Q projection (collective mode: after V so collectives overlap) ----
            if collective:
                for qtr, tin in proj_quarters(qT, SQ):
                    for half in range(2):
                        cb = qtr * 2 + half
                        proj512(wq_sb, tin, half, QT[:, cb * 512:(cb + 1) * 512])

            # ---- masks ----
            mska_sb = constp.tile([128, 128], BF16)
            nc.sync.dma_start(mska_sb[:], mska[:])
            mskb_sb = constp.tile([128, 128], BF16)
            nc.sync.dma_start(mskb_sb[:], mskb[:])
            zbias = constp.tile([128, 1], F32)
            nc.vector.memset(zbias[:], 0.0)

            # ---- unpack gathered K/V ----
            if collective:
                for r in range(2):
                    nc.sync.dma_start(
                        KT[:, r * SK:(r + 1) * SK], cc_out_k[r]
                    )
                    nc.sync.dma_start(
                        VA[:, r * nloc_kt:(r + 1) * nloc_kt, 0:DV],
                        cc_out_v[r].rearrange("p (kt d) -> p kt d", d=DV),
                    )

            # ---- attention, per q-chunk of 512 ----
            for cc in range(NCH):
                npair = 4 * cc + 4
                PT = ptp.tile([128, MAXKT, 512], BF16, tag="pt")
                for a in range(npair):
                    j0 = max(0, a - 4 * cc)
                    n = (4 - j0) * 128
                    qoff = cc * 512 + j0 * 128
                    st = pstp.tile([128, 2, 512], F32, tag="stps")
                    for half in range(2):
                        kt = 2 * a + half
                        nc.tensor.matmul(
                            st[:, half, :n],
                            KT[:, kt * 128:(kt + 1) * 128],
                            QT[:, qoff:qoff + n],
                            start=True, stop=True,
                        )
                    nc.scalar.activation(
                        PT[:, 2 * a:2 * a + 2, j0 * 128:512],
                        st[:, :, :n],
                        Exp, bias=zbias[:],
                    )
                    for j in range(j0, 4):
                        i = 4 * cc + j
                        for half in range(2):
                            kt = 2 * a + half
                            msk = None
                            if kt == 2 * i:
                                msk = mska_sb
                            elif kt == 2 * i + 1:
                                msk = mskb_sb
                            if msk is not None:
                                sl = PT[:, kt, j * 128:(j + 1) * 128]
                                nc.vector.tensor_mul(sl, sl, msk[:])

                for j in range(4):
                    i = 4 * cc + j
                    nkt_i = 2 * i + 2
                    po = psop.tile([128, DV + 1], F32, tag="ops")
                    for kt in range(nkt_i):
                        nc.tensor.matmul(
                            po[:], PT[:, kt, j * 128:(j + 1) * 128], VA[:, kt, 0:DV + 1],
                            start=(kt == 0), stop=(kt == nkt_i - 1),
                        )
                    rec = outp.tile([128, 1], F32, tag="rec")
                    nc.vector.reciprocal(rec[:], po[:, DV:DV + 1])
                    ob = outp.tile([128, DV], F32, tag="ob")
                    nc.vector.tensor_scalar_mul(ob[:], po[:, 0:DV], rec[:])
                    nc.sync.dma_start(out[i * 128:(i + 1) * 128, :], ob[:])

    nc.compile()
    return nc


def build_nc_v3():
    """v3: keys split by tile parity within each batch pair; K^T and V are
    exchanged with *segmented* AllGathers (K: 2 segs, V: 3 segs) so attention
    score chunks start as soon as their key range has landed.  Attention is
    interleaved into the projection stream: the PE instruction order follows
    expected data-arrival order so no engine waits on late data.

    Layouts: KT4 [128(dk), r, j, 128] with global key tile g = 2j + r
    (r = producing rank parity), VA [128(key), r, j, DV+2] with ones column
    at [:, :, :, DV] for the softmax denominator."""
    NLT = SK // 128            # 16 local key tiles per core
    RG = [[0, 1], [2, 3], [4, 5], [6, 7]]

    nc = bacc.Bacc(None, target_bir_lowering=False, num_devices=8)

    qT = nc.declare_dram_parameter("qT", [DM, SQ], BF16, isOutput=False)
    kT = nc.declare_dram_parameter("kT", [DM, SK], BF16, isOutput=False)
    vT = nc.declare_dram_parameter("vT", [DM, SK], BF16, isOutput=False)
    wq = nc.declare_dram_parameter("wq", [128, NMC * DK], BF16, isOutput=False)
    wk = nc.declare_dram_parameter("wk", [128, NMC * DK], BF16, isOutput=False)
    wv = nc.declare_dram_parameter("wv", [128, NMC * DV], BF16, isOutput=False)
    mska = nc.declare_dram_parameter("mska", [128, 128], BF16, isOutput=False)
    mskb = nc.declare_dram_parameter("mskb", [128, 128], BF16, isOutput=False)
    out = nc.declare_dram_parameter("out", [SQ, DV], F32, isOutput=True)

    Exp = mybir.ActivationFunctionType.Exp

    with tile.TileContext(nc) as tc:
        with (
            tc.tile_pool(name="const", bufs=1) as constp,
            tc.tile_pool(name="stream", bufs=3) as streamp,
            tc.tile_pool(name="big", bufs=1) as bigp,
            tc.tile_pool(name="ptp", bufs=2) as ptp,
            tc.tile_pool(name="outp", bufs=4) as outp,
            tc.tile_pool(name="dram", bufs=1, space="DRAM") as dramp,
            tc.tile_pool(name="ps", bufs=2, space="PSUM") as psp,
            tc.tile_pool(name="pst", bufs=2, space="PSUM") as pstp,
            tc.tile_pool(name="pso", bufs=2, space="PSUM") as psop,
        ):
            # ---- constants (wk first: K projection starts earliest) ----
            wk_sb = constp.tile([128, NMC, DK], BF16)
            nc.sync.dma_start(wk_sb[:], wk.rearrange("p (mc d) -> p mc d", d=DK))
            mska_sb = constp.tile([128, 128], BF16)
            nc.sync.dma_start(mska_sb[:], mska[:])
            mskb_sb = constp.tile([128, 128], BF16)
            nc.sync.dma_start(mskb_sb[:], mskb[:])
            wq_sb = constp.tile([128, NMC, DK], BF16)
            nc.sync.dma_start(wq_sb[:], wq.rearrange("p (mc d) -> p mc d", d=DK))
            wv_sb = constp.tile([128, NMC, DV], BF16)
            nc.sync.dma_start(wv_sb[:], wv.rearrange("p (mc d) -> p mc d", d=DV))

            zbias = constp.tile([128, 1], F32)
            nc.vector.memset(zbias[:], 0.0)

            # ---- persistent activations ----
            QT = bigp.tile([128, SQ], BF16)
            KT_loc = bigp.tile([128, SK], BF16)
            V_loc = bigp.tile([128, NLT, DV], BF16)
            KT4 = bigp.tile([128, 2, NLT, 128], BF16)
            VA = bigp.tile([128, 2, NLT, DV + 2], BF16)
            nc.vector.memset(VA[:], 1.0)  # ones at [:, :, :, DV]

            # ---- PE warm-up (HAM to 8/8 before first projection) ----
            warm = constp.tile([128, 256], BF16)
            nc.vector.memset(warm[:], 0.0)
            wps = psp.tile([128, 256], F32, tag="projps")
            for _ in range(20):
                nc.tensor.matmul(wps[:], warm[:, 0:128], warm[:], start=True, stop=True)

            # ---- collective buffers ----
            KSEG = [(0, 8), (8, 8)]            # (j0, nj) local-tile ranges
            VSEG = [(0, 8), (8, 4), (12, 4)]
            cc_in_k = [dramp.tile([128, nj * 128], BF16, name=f"cik{s}")
                       for s, (j0, nj) in enumerate(KSEG)]
            cc_out_k = [dramp.tile([2, 128, nj * 128], BF16, name=f"cok{s}")
                        for s, (j0, nj) in enumerate(KSEG)]
            cc_in_v = [dramp.tile([128, nj * DV], BF16, name=f"civ{s}")
                       for s, (j0, nj) in enumerate(VSEG)]
            cc_out_v = [dramp.tile([2, 128, nj * DV], BF16, name=f"cov{s}")
                        for s, (j0, nj) in enumerate(VSEG)]

            def ag_k(s):
                j0, nj = KSEG[s]
                nc.sync.dma_start(cc_in_k[s][:], KT_loc[:, j0 * 128:(j0 + nj) * 128])
                nc.gpsimd.collective_compute(
                    "AllGather", mybir.AluOpType.bypass, replica_groups=RG,
                    ins=[cc_in_k[s][:]], outs=[cc_out_k[s][:]],
                )
                for r in range(2):
                    nc.sync.dma_start(
                        KT4[:, r, j0:j0 + nj, :],
                        cc_out_k[s][r].rearrange("p (j c) -> p j c", c=128),
                    )

            def ag_v(s):
                j0, nj = VSEG[s]
                nc.sync.dma_start(cc_in_v[s][:], V_loc[:, j0:j0 + nj, :])
                nc.gpsimd.collective_compute(
                    "AllGather", mybir.AluOpType.bypass, replica_groups=RG,
                    ins=[cc_in_v[s][:]], outs=[cc_out_v[s][:]],
                )
                for r in range(2):
                    nc.sync.dma_start(
                        VA[:, r, j0:j0 + nj, 0:DV],
                        cc_out_v[s][r].rearrange("p (j d) -> p j d", d=DV),
                    )

            # ---- streamed projections (512-col chunks) ----
            def stream_chunk(src_dram, c):
                tin = streamp.tile([128, NMC, 512], BF16, tag="instream")
                nc.sync.dma_start(
                    tin[:],
                    src_dram[:, c * 512:(c + 1) * 512].rearrange(
                        "(mc p) c -> p mc c", p=128
                    ),
                )
                return tin

            def proj512(w_sb, tin, dst_sb_slice):
                ps = psp.tile([128, 512], F32, tag="projps")
                for m in range(NMC):
                    nc.tensor.matmul(
                        ps[:], w_sb[:, m, :], tin[:, m, :],
                        start=(m == 0), stop=(m == NMC - 1),
                    )
                nc.vector.tensor_copy(dst_sb_slice, ps[:])

            def vproj_chunk(tin, c):
                for sl in range(4):
                    lt = c * 4 + sl
                    vps = psp.tile([128, DV], F32, tag="projps")
                    for m in range(NMC):
                        nc.tensor.matmul(
                            vps[:], tin[:, m, sl * 128:(sl + 1) * 128], wv_sb[:, m, :],
                            start=(m == 0), stop=(m == NMC - 1),
                        )
                    nc.vector.tensor_copy(V_loc[:, lt, :], vps[:])

            # ---- attention pieces ----
            PT_tiles = {}

            def scores_chunk(cc):
                npair = 4 * cc + 4
                PT = ptp.tile([128, MAXKT, 512], BF16, tag="pt")
                PT_tiles[cc] = PT
                for a in range(npair):
                    j0 = max(0, a - 4 * cc)
                    n = (4 - j0) * 128
                    qoff = cc * 512 + j0 * 128
                    st = pstp.tile([128, 2, 512], F32, tag="stps")
                    for half in range(2):
                        kt = 2 * a + half
                        nc.tensor.matmul(
                            st[:, half, :n],
                            KT4[:, kt % 2, kt // 2, :],
                            QT[:, qoff:qoff + n],
                            start=True, stop=True,
                        )
                    nc.scalar.activation(
                        PT[:, 2 * a:2 * a + 2, j0 * 128:512],
                        st[:, :, :n],
                        Exp, bias=zbias[:],
                    )
                    for j in range(j0, 4):
                        i = 4 * cc + j
                        for half in range(2):
                            kt = 2 * a + half
                            msk = None
                            if kt == 2 * i:
                                msk = mska_sb
                            elif kt == 2 * i + 1:
                                msk = mskb_sb
                            if msk is not None:
                                sl = PT[:, kt, j * 128:(j + 1) * 128]
                                nc.vector.tensor_mul(sl, sl, msk[:])

            def pv_chunk(cc):
                PT = PT_tiles[cc]
                for j in range(4):
                    i = 4 * cc + j
                    nkt_i = 2 * i + 2
                    po = psop.tile([128, DV + 1], F32, tag="ops")
                    for kt in range(nkt_i):
                        nc.tensor.matmul(
                            po[:], PT[:, kt, j * 128:(j + 1) * 128],
                            VA[:, kt % 2, kt // 2, 0:DV + 1],
                            start=(kt == 0), stop=(kt == nkt_i - 1),
                        )
                    rec = outp.tile([128, 1], F32, tag="rec")
                    nc.vector.reciprocal(rec[:], po[:, DV:DV + 1])
                    ob = outp.tile([128, DV], F32, tag="ob")
                    nc.vector.tensor_scalar_mul(ob[:], po[:, 0:DV], rec[:])
                    nc.sync.dma_start(out[i * 128:(i + 1) * 128, :], ob[:])

            # ---- the interleaved schedule ----
            # arrival/PE order: k0 q0 k1 q1 | sc0 sc1 | k2 k3 q2 sc2 |
            #                   v0 v1 q3 sc3 | v2 v3 | pv0..pv3
            def kproj(c):
                tin = stream_chunk(kT, c)
                proj512(wk_sb, tin, KT_loc[:, c * 512:(c + 1) * 512])

            def qproj(c):
                tin = stream_chunk(qT, c)
                proj512(wq_sb, tin, QT[:, c * 512:(c + 1) * 512])

            kproj(0)
            qproj(0)
            kproj(1)
            ag_k(0)
            qproj(1)
            scores_chunk(0)
            scores_chunk(1)
            kproj(2)
            kproj(3)
            ag_k(1)
            qproj(2)
            vproj_chunk(stream_chunk(vT, 0), 0)
            scores_chunk(2)
            vproj_chunk(stream_chunk(vT, 1), 1)
            ag_v(0)
            qproj(3)
            scores_chunk(3)
            vproj_chunk(stream_chunk(vT, 2), 2)
            ag_v(1)
            vproj_chunk(stream_chunk(vT, 3), 3)
            ag_v(2)
            pv_chunk(0)
            pv_chunk(1)
            pv_chunk(2)
            pv_chunk(3)

    nc.compile()
    return nc


def build_nc_v4():
    """v4 = v3 with the trigger-queue serialization fixed:

    - all collective staging (SBUF->DRAM) and unpack (DRAM->SBUF) DMAs run on
      the GpSimd (SWDGE) queue, whose in-order semantics match their true
      dependencies, leaving the Sync queue a pure linear input/output stream;
    - 1024-col input chunks (2 KB DMA lines, half the trigger count);
    - weights and masks merged into single params (2 const DMAs);
    - outputs staged per 512-query chunk (4 output DMAs instead of 16);
    - V carries its denominator ones-columns through the AllGather so the
      unpack is a single contiguous DMA."""
    NLT = SK // 128            # 16 local key tiles per core
    DVP = DV + 2               # V row padded with ones at [DV] (and [DV+1])
    RG = [[0, 1], [2, 3], [4, 5], [6, 7]]

    nc = bacc.Bacc(None, target_bir_lowering=False, num_devices=8)

    qT = nc.declare_dram_parameter("qT", [DM, SQ], BF16, isOutput=False)
    kT = nc.declare_dram_parameter("kT", [DM, SK], BF16, isOutput=False)
    vT = nc.declare_dram_parameter("vT", [DM, SK], BF16, isOutput=False)
    w = nc.declare_dram_parameter("w", [128, 3 * NMC * DK], BF16, isOutput=False)
    msk = nc.declare_dram_parameter("msk", [128, 256], BF16, isOutput=False)
    out = nc.declare_dram_parameter("out", [SQ, DV], F32, isOutput=True)

    Exp = mybir.ActivationFunctionType.Exp

    with tile.TileContext(nc) as tc:
        with (
            tc.tile_pool(name="const", bufs=1) as constp,
            tc.tile_pool(name="stream", bufs=3) as streamp,
            tc.tile_pool(name="big", bufs=1) as bigp,
            tc.tile_pool(name="ptp", bufs=2) as ptp,
            tc.tile_pool(name="outp", bufs=4) as outp,
            tc.tile_pool(name="dram", bufs=1, space="DRAM") as dramp,
            tc.tile_pool(name="ps", bufs=2, space="PSUM") as psp,
            tc.tile_pool(name="pst", bufs=2, space="PSUM") as pstp,
            tc.tile_pool(name="pso", bufs=2, space="PSUM") as psop,
        ):
            # ---- consts ----
            w_sb = constp.tile([128, 3, NMC, DK], BF16)
            nc.sync.dma_start(
                w_sb[:], w.rearrange("p (t mc d) -> p t mc d", mc=NMC, d=DK)
            )
            msk_sb = constp.tile([128, 2, 128], BF16)
            nc.sync.dma_start(msk_sb[:], msk.rearrange("p (t c) -> p t c", c=128))
            zbias = constp.tile([128, 1], F32)
            nc.vector.memset(zbias[:], 0.0)

            # ---- persistent activations ----
            QT = bigp.tile([128, SQ], BF16)
            KT_loc = bigp.tile([128, SK], BF16)
            V_loc = bigp.tile([128, NLT, DVP], BF16)
            nc.vector.memset(V_loc[:], 1.0)  # ones at [:, :, DV:]
            KT4 = bigp.tile([128, 2, NLT, 128], BF16)
            VA = bigp.tile([128, 2, NLT, DVP], BF16)

            # ---- PE warm-up ----
            warm = constp.tile([128, 256], BF16)
            nc.vector.memset(warm[:], 0.0)
            wps = psp.tile([128, 256], F32, tag="projps")
            for _ in range(30):
                nc.tensor.matmul(wps[:], warm[:, 0:128], warm[:], start=True, stop=True)

            # ---- collective buffers (segments over local-tile index j) ----
            KSEG = [(0, 8), (8, 8)]
            VSEG = [(0, 8), (8, 4), (12, 4)]
            cc_in_k = [dramp.tile([128, nj * 128], BF16, name=f"cik{s}")
                       for s, (j0, nj) in enumerate(KSEG)]
            cc_out_k = [dramp.tile([2, 128, nj * 128], BF16, name=f"cok{s}")
                        for s, (j0, nj) in enumerate(KSEG)]
            cc_in_v = [dramp.tile([128, nj * DVP], BF16, name=f"civ{s}")
                       for s, (j0, nj) in enumerate(VSEG)]
            cc_out_v = [dramp.tile([2, 128, nj * DVP], BF16, name=f"cov{s}")
                        for s, (j0, nj) in enumerate(VSEG)]

            # Queue map (each engine's stream ordered to match true deps):
            #   scalar: cik/upk interleaved with the exps that consume KT4
            #   gpsimd: AllGathers + civ staging (idle otherwise)
            #   sync:   input/output streams + upv (free after inputs drain)
            def ag_k(s):
                j0, nj = KSEG[s]
                nc.scalar.dma_start(cc_in_k[s][:], KT_loc[:, j0 * 128:(j0 + nj) * 128])
                nc.gpsimd.collective_compute(
                    "AllGather", mybir.AluOpType.bypass, replica_groups=RG,
                    ins=[cc_in_k[s][:]], outs=[cc_out_k[s][:]],
                )
                nc.scalar.dma_start(
                    KT4[:, :, j0:j0 + nj, :],
                    cc_out_k[s].rearrange("r p (j c) -> p r j c", c=128),
                )

            def ag_v(s):
                j0, nj = VSEG[s]
                nc.gpsimd.dma_start(cc_in_v[s][:], V_loc[:, j0:j0 + nj, :])
                nc.gpsimd.collective_compute(
                    "AllGather", mybir.AluOpType.bypass, replica_groups=RG,
                    ins=[cc_in_v[s][:]], outs=[cc_out_v[s][:]],
                )
                nc.sync.dma_start(
                    VA[:, :, j0:j0 + nj, :],
                    cc_out_v[s].rearrange("r p (j d) -> p r j d", d=DVP),
                )

            # ---- streamed projections (1024-col chunks) ----
            def stream_chunk(src_dram, c):
                tin = streamp.tile([128, NMC, 1024], BF16, tag="instream")
                nc.sync.dma_start(
                    tin[:],
                    src_dram[:, c * 1024:(c + 1) * 1024].rearrange(
                        "(mc p) c -> p mc c", p=128
                    ),
                )
                return tin

            def proj512(wi, tin, half, dst_sb_slice):
                ps = psp.tile([128, 512], F32, tag="projps")
                for m in range(NMC):
                    nc.tensor.matmul(
                        ps[:], w_sb[:, wi, m, :], tin[:, m, half * 512:(half + 1) * 512],
                        start=(m == 0), stop=(m == NMC - 1),
                    )
                nc.vector.tensor_copy(dst_sb_slice, ps[:])

            def kproj(c):  # c in 0..1, covers local tiles 8c..8c+7
                tin = stream_chunk(kT, c)
                for half in range(2):
                    cb = 2 * c + half
                    proj512(1, tin, half, KT_loc[:, cb * 512:(cb + 1) * 512])

            def qproj(c):  # c in 0..1, covers q chunks 2c..2c+1
                tin = stream_chunk(qT, c)
                for half in range(2):
                    cb = 2 * c + half
                    proj512(0, tin, half, QT[:, cb * 512:(cb + 1) * 512])

            def vproj4(tin, c, half):  # 4 local tiles c*8+half*4 ..+3
                for sl in range(4 * half, 4 * half + 4):
                    lt = c * 8 + sl
                    vps = psp.tile([128, DV], F32, tag="projps")
                    for m in range(NMC):
                        nc.tensor.matmul(
                            vps[:], tin[:, m, sl * 128:(sl + 1) * 128],
                            w_sb[:, 2, m, :],
                            start=(m == 0), stop=(m == NMC - 1),
                        )
                    nc.vector.tensor_copy(V_loc[:, lt, 0:DV], vps[:])

            # ---- attention ----
            PT_tiles = {}

            def scores_chunk(cc):
                npair = 4 * cc + 4
                PT = ptp.tile([128, MAXKT, 512], BF16, tag="pt")
                PT_tiles[cc] = PT
                for a in range(npair):
                    j0 = max(0, a - 4 * cc)
                    n = (4 - j0) * 128
                    qoff = cc * 512 + j0 * 128
                    st = pstp.tile([128, 2, 512], F32, tag="stps")
                    for half in range(2):
                        kt = 2 * a + half
                        nc.tensor.matmul(
                            st[:, half, :n],
                            KT4[:, kt % 2, kt // 2, :],
                            QT[:, qoff:qoff + n],
                            start=True, stop=True,
                        )
                    nc.scalar.activation(
                        PT[:, 2 * a:2 * a + 2, j0 * 128:512],
                        st[:, :, :n],
                        Exp, bias=zbias[:],
                    )
                    for j in range(j0, 4):
                        i = 4 * cc + j
                        for half in range(2):
                            kt = 2 * a + half
                            mi = None
                            if kt == 2 * i:
                                mi = 0
                            elif kt == 2 * i + 1:
                                mi = 1
                            if mi is not None:
                                sl = PT[:, kt, j * 128:(j + 1) * 128]
                                nc.vector.tensor_mul(sl, sl, msk_sb[:, mi, :])

            def pv_chunk(cc):
                PT = PT_tiles[cc]
                ostg = outp.tile([128, 4, DV], F32, tag="ostg")
                for j in range(4):
                    i = 4 * cc + j
                    nkt_i = 2 * i + 2
                    po = psop.tile([128, DV + 1], F32, tag="ops")
                    for kt in range(nkt_i):
                        nc.tensor.matmul(
                            po[:], PT[:, kt, j * 128:(j + 1) * 128],
                            VA[:, kt % 2, kt // 2, 0:DV + 1],
                            start=(kt == 0), stop=(kt == nkt_i - 1),
                        )
                    rec = outp.tile([128, 1], F32, tag="rec")
                    nc.vector.reciprocal(rec[:], po[:, DV:DV + 1])
                    nc.vector.tensor_scalar_mul(ostg[:, j, :], po[:, 0:DV], rec[:])
                nc.sync.dma_start(
                    out[cc * 512:(cc + 1) * 512, :].rearrange("(t p) d -> p t d", p=128),
                    ostg[:],
                )

            # ---- schedule ----
            kproj(0)
            ag_k(0)
            qproj(0)
            scores_chunk(0)
            scores_chunk(1)
            kproj(1)
            ag_k(1)
            qproj(1)
            scores_chunk(2)
            scores_chunk(3)
            tin_v0 = stream_chunk(vT, 0)
            vproj4(tin_v0, 0, 0)
            vproj4(tin_v0, 0, 1)
            ag_v(0)
            tin_v1 = stream_chunk(vT, 1)
            vproj4(tin_v1, 1, 0)
            ag_v(1)
            vproj4(tin_v1, 1, 1)
            ag_v(2)
            pv_chunk(0)
            pv_chunk(1)
            pv_chunk(2)
            pv_chunk(3)

    nc.compile()
    return nc


def build_nc_v6():
    """v6: no collectives (measured: cc DMAs starve ~20x against the bulk
    input stream, and DRAM round-trips cost as much as the duplicated
    projections they replace).  Each core projects its batch's full K/V;
    attention chunks are interleaved into the projection stream so the PE
    never phase-waits.  Stream order feeds the exp (ScalarE) path first:
    k0 q0 k1 q1 k2 k3 v0..v3."""
    RG = None  # no collectives
    DVP = DV + 2

    nc = bacc.Bacc(None, target_bir_lowering=False, num_devices=8)

    qT = nc.declare_dram_parameter("qT", [DM, SQ], BF16, isOutput=False)
    kT = nc.declare_dram_parameter("kT", [DM, S], BF16, isOutput=False)
    vT = nc.declare_dram_parameter("vT", [DM, S], BF16, isOutput=False)
    w = nc.declare_dram_parameter("w", [128, 3 * NMC * DK], BF16, isOutput=False)
    msk = nc.declare_dram_parameter("msk", [128, 256], BF16, isOutput=False)
    out = nc.declare_dram_parameter("out", [SQ, DV], F32, isOutput=True)

    Exp = mybir.ActivationFunctionType.Exp

    with tile.TileContext(nc) as tc:
        with (
            tc.tile_pool(name="const", bufs=1) as constp,
            tc.tile_pool(name="stream", bufs=5) as streamp,
            tc.tile_pool(name="big", bufs=1) as bigp,
            tc.tile_pool(name="ptp", bufs=2) as ptp,
            tc.tile_pool(name="outp", bufs=4) as outp,
            tc.tile_pool(name="ps", bufs=2, space="PSUM") as psp,
            tc.tile_pool(name="pst", bufs=2, space="PSUM") as pstp,
            tc.tile_pool(name="pso", bufs=2, space="PSUM") as psop,
        ):
            w_sb = constp.tile([128, 3, NMC, DK], BF16)
            nc.sync.dma_start(
                w_sb[:], w.rearrange("p (t mc d) -> p t mc d", mc=NMC, d=DK)
            )
            msk_sb = constp.tile([128, 2, 128], BF16)
            nc.sync.dma_start(msk_sb[:], msk.rearrange("p (t c) -> p t c", c=128))

            # warm-up tile memset FIRST on DVE so the PE warm-up isn't gated
            # behind the big VA fill
            warm = constp.tile([128, 256], BF16)
            nc.vector.memset(warm[:], 0.0)
            wps = psp.tile([128, 256], F32, tag="projps")
            for _ in range(12):
                nc.tensor.matmul(wps[:], warm[:, 0:128], warm[:], start=True, stop=True)

            zbias = constp.tile([128, 1], F32)
            nc.vector.memset(zbias[:], 0.0)

            QT = bigp.tile([128, SQ], BF16)
            KT = bigp.tile([128, S], BF16)
            VA = bigp.tile([128, MAXKT, DVP], BF16)
            nc.vector.memset(VA[:], 1.0)

            def stream_chunk(src_dram, c0, ncols):
                nmcv = ncols  # free-dim cols of this chunk
                tin = streamp.tile([128, NMC, nmcv], BF16, tag="instream",
                                   name=f"in{c0}_{ncols}")
                nc.sync.dma_start(
                    tin[:],
                    src_dram[:, c0:c0 + ncols].rearrange("(mc p) c -> p mc c", p=128),
                )
                return tin

            def proj512(wi, tin, half, dst_sb_slice):
                ps = psp.tile([128, 512], F32, tag="projps")
                for m in range(NMC):
                    nc.tensor.matmul(
                        ps[:], w_sb[:, wi, m, :], tin[:, m, half * 512:(half + 1) * 512],
                        start=(m == 0), stop=(m == NMC - 1),
                    )
                nc.vector.tensor_copy(dst_sb_slice, ps[:])

            def kproj(c0, ncols):  # projects key cols [c0, c0+ncols)
                tin = stream_chunk(kT, c0, ncols)
                for half in range(ncols // 512):
                    cb = c0 // 512 + half
                    proj512(1, tin, half, KT[:, cb * 512:(cb + 1) * 512])

            def qproj(c0, ncols):
                tin = stream_chunk(qT, c0, ncols)
                for half in range(ncols // 512):
                    cb = c0 // 512 + half
                    proj512(0, tin, half, QT[:, cb * 512:(cb + 1) * 512])

            def vproj(c):  # 1024-col chunk c -> key tiles 8c..8c+7
                tin = stream_chunk(vT, c * 1024, 1024)
                for sl in range(8):
                    lt = c * 8 + sl
                    vps = psp.tile([128, DV], F32, tag="projps")
                    for m in range(NMC):
                        nc.tensor.matmul(
                            vps[:], tin[:, m, sl * 128:(sl + 1) * 128],
                            w_sb[:, 2, m, :],
                            start=(m == 0), stop=(m == NMC - 1),
                        )
                    nc.vector.tensor_copy(VA[:, lt, 0:DV], vps[:])

            PT_tiles = {}

            def scores_chunk(cc):
                npair = 4 * cc + 4
                PT = ptp.tile([128, MAXKT, 512], BF16, tag="pt")
                PT_tiles[cc] = PT
                for a in range(npair):
                    j0 = max(0, a - 4 * cc)
                    n = (4 - j0) * 128
                    qoff = cc * 512 + j0 * 128
                    st = pstp.tile([128, 2, 512], F32, tag="stps")
                    for half in range(2):
                        kt = 2 * a + half
                        nc.tensor.matmul(
                            st[:, half, :n],
                            KT[:, kt * 128:(kt + 1) * 128],
                            QT[:, qoff:qoff + n],
                            start=True, stop=True,
                        )
                    nc.scalar.activation(
                        PT[:, 2 * a:2 * a + 2, j0 * 128:512],
                        st[:, :, :n],
                        Exp, bias=zbias[:],
                    )
                    for j in range(j0, 4):
                        i = 4 * cc + j
                        for half in range(2):
                            kt = 2 * a + half
                            mi = 0 if kt == 2 * i else (1 if kt == 2 * i + 1 else None)
                            if mi is not None:
                                sl = PT[:, kt, j * 128:(j + 1) * 128]
                                nc.vector.tensor_mul(sl, sl, msk_sb[:, mi, :])

            def pv_chunk(cc):
                PT = PT_tiles[cc]
                ostg = outp.tile([128, 4, DV], F32, tag="ostg")
                for j in range(4):
                    i = 4 * cc + j
                    nkt_i = 2 * i + 2
                    po = psop.tile([128, DV + 1], F32, tag="ops")
                    for kt in range(nkt_i):
                        nc.tensor.matmul(
                            po[:], PT[:, kt, j * 128:(j + 1) * 128],
                            VA[:, kt, 0:DV + 1],
                            start=(kt == 0), stop=(kt == nkt_i - 1),
                        )
                    rec = outp.tile([128, 1], F32, tag="rec")
                    nc.vector.reciprocal(rec[:], po[:, DV:DV + 1])
                    nc.vector.tensor_scalar_mul(ostg[:, j, :], po[:, 0:DV], rec[:])
                nc.sync.dma_start(
                    out[cc * 512:(cc + 1) * 512, :].rearrange("(t p) d -> p t d", p=128),
                    ostg[:],
                )

            # schedule: stream order = w msk k0a k0b q0 k1 q1 k2 k3 v0 v1 v2 v3
            kproj(0, 512)
            kproj(512, 512)
            qproj(0, 1024)
            scores_chunk(0)
            kproj(1024, 1024)
            scores_chunk(1)
            qproj(1024, 1024)
            kproj(2048, 1024)
            scores_chunk(2)
            kproj(3072, 1024)
            scores_chunk(3)
            vproj(0)
            vproj(1)
            pv_chunk(0)
            pv_chunk(1)
            vproj(2)
            pv_chunk(2)
            vproj(3)
            pv_chunk(3)

    nc.compile()
    return nc


def _prep_inputs(q, k, v, W_Q, W_K, W_V, mode=None):
    mode = MODE if mode is None else mode
    collective = mode == "v2"
    q = np.asarray(q, dtype=np.float32)
    k = np.asarray(k, dtype=np.float32)
    v = np.asarray(v, dtype=np.float32)
    W_Q = np.asarray(W_Q, dtype=np.float32)
    W_K = np.asarray(W_K, dtype=np.float32)
    W_V = np.asarray(W_V, dtype=np.float32)

    scale = 1.0 / math.sqrt(DK)

    def wtile(w):  # [128, 1024] pre-tiled: row p, cols mc*128+d
        return np.ascontiguousarray(
            w.T.reshape(NMC, 128, w.shape[0]).transpose(1, 0, 2).reshape(128, NMC * w.shape[0])
        ).astype(BF16NP)

    wq_h = wtile(W_Q * scale)
    wk_h = wtile(W_K)
    wv_h = wtile(W_V)
    tri = np.triu(np.ones((128, 128), np.float32)).astype(BF16NP)
    ones = np.ones((128, 128), BF16NP)
    zeros = np.zeros((128, 128), BF16NP)

    in_maps = []
    for c in range(8):
        b, p = c // 2, c % 2
        idx = np.arange(NQT) * 2 + p
        qsel = q[b].reshape(MAXKT, 128, DM)[idx].reshape(SQ, DM)
        if mode == "v6":
            kslc = k[b]
            vslc = v[b]
        elif mode in ("v3", "v4"):
            kidx = np.arange(SK // 128) * 2 + p
            kslc = k[b].reshape(MAXKT, 128, DM)[kidx].reshape(SK, DM)
            vslc = v[b].reshape(MAXKT, 128, DM)[kidx].reshape(SK, DM)
        elif collective:
            kslc = k[b, p * SK:(p + 1) * SK, :]
            vslc = v[b, p * SK:(p + 1) * SK, :]
        else:
            kslc = k[b]
            vslc = v[b]
        im = {
            "qT": np.ascontiguousarray(qsel.T).astype(BF16NP),
            "kT": np.ascontiguousarray(kslc.T).astype(BF16NP),
            "vT": np.ascontiguousarray(vslc.T).astype(BF16NP),
        }
        mska_h = ones if p == 1 else tri
        mskb_h = tri if p == 1 else zeros
        if mode in ("v4", "v6"):
            im["w"] = np.ascontiguousarray(
                np.concatenate([wq_h, wk_h, wv_h], axis=1)
            )
            im["msk"] = np.ascontiguousarray(
                np.concatenate([mska_h, mskb_h], axis=1)
            )
        else:
            im.update({
                "wq": wq_h, "wk": wk_h, "wv": wv_h,
                "mska": mska_h, "mskb": mskb_h,
            })
            if mode != "v3":
                im["eye"] = np.eye(128, dtype=np.float32).astype(BF16NP)
        in_maps.append(im)
    return in_maps


def kernel(q, k, v, W_Q, W_K, W_V):
    global LAST_RESULTS
    if MODE not in _NC_CACHE:
        if MODE == "v6":
            _NC_CACHE[MODE] = build_nc_v6()
        elif MODE == "v4":
            _NC_CACHE[MODE] = build_nc_v4()
        elif MODE == "v3":
            _NC_CACHE[MODE] = build_nc_v3()
        else:
            _NC_CACHE[MODE] = build_nc(MODE)
    nc = _NC_CACHE[MODE]

    in_maps = _prep_inputs(q, k, v, W_Q, W_K, W_V)
    res = run_bass_kernel_spmd(nc, in_maps, core_ids=list(range(8)))
    LAST_RESULTS = res

    out = np.empty((B, S, DV), np.float32)
    for c in range(8):
        b, p = c // 2, c % 2
        oc = res.results[c]["out"]
        out[b].reshape(MAXKT, 128, DV)[np.arange(NQT) * 2 + p] = (
            oc.reshape(NQT, 128, DV)
        )
    return out

